# revision 3
# baseline (speedup 1.0000x reference)
# GQA attention (B=2, T=2048, DM=2048, H=16, KV=4, D=128) + RoPE + causal mask
# on 8 TRN2 NeuronCores.
#
# Sharding: rank r = (batch b = r//4, kv-group g = r%4).  Each rank computes
# q-heads 4g..4g+3 and kv-head g for batch b (full sequence), does the SDPA
# head-sharded, then AllGathers attention outputs within each 4-rank batch
# group (chunked, overlapped with later attention).  The o_proj is
# column-sharded: each rank multiplies the gathered O^T by its 512-column
# slice of Wo, accumulating in SBUF as AllGather chunks arrive, and returns
# o^T [512, 2048]; the host transposes and concatenates.
#
# v2 structure (vs the phase-serial baseline):
#  - ONE pool scope: projection and attention tiles coexist, so the Tile
#    scheduler can overlap them (the baseline's scoped pools aliased
#    addresses, serializing attention behind the last q projection).
#  - q-head h+1's projection pairs are emitted interleaved with head h's
#    attention chunks, giving the PE queue scalar-independent work while
#    ScalarE's exp stream catches up.
#  - softmax reciprocal = exp(-ln(x)) and a patched activation-table map
#    make the whole kernel use ONE table set (the baseline ping-ponged
#    exp<->reciprocal tables, 18 reloads x ~1.3us).
#  - rope's rotate-half partition swaps ride the GpSimd DMA queue; weight /
#    x / output DMAs are split across the scalar and sync queues; weights
#    are pre-rearranged on the host so every load is contiguous.

import functools
import os
import sys

import numpy as np

for _p in ("/opt/trn_rl_repo", "/root/.axon_site/_ro/trn_rl_repo"):
    if os.path.isdir(_p) and _p not in sys.path:
        sys.path.insert(0, _p)

import ml_dtypes

import concourse.bass as bass
import concourse.mybir as mybir
import concourse.tile as tile
import concourse.masks as masks
from concourse import bacc
import concourse.hw_specs as hw_specs
from concourse.bass_utils import run_bass_kernel_spmd

BF16 = ml_dtypes.bfloat16

B, T, DM = 2, 2048, 2048
H, KV, D = 16, 4, 128
NH = H // KV  # 4 local q heads per rank
P = 128
NCORES = 8
NT = T // 512  # 4 free-dim chunks of 512
NDC = DM // P  # 16 contraction chunks
SCALE = float(D) ** -0.5
ROPE_BASE = 10000.0

_bf = mybir.dt.bfloat16
_f32 = mybir.dt.float32

# AllGather chunk groups per head: head 3's tail is split finer so the last
# (fully exposed) collective is as small as possible
GROUPS = {h: ([0, 1], [2, 3]) if h < NH - 1 else ([0, 1], [2], [3]) for h in range(NH)}


@functools.cache
def _patched_tables(arch):
    # The act-table insertion pass maps each ActivationFunctionType to the
    # first table set containing it, so alternating Exp and Ln would reload
    # tables on every switch even though natural_log_exp_and_others holds
    # both.  Strip Exp/Ln from every other set (keeping list positions, which
    # are the runtime set ids) so both resolve to the combined set.
    orig = hw_specs.get_activation_tables(arch)
    exp_t = mybir.ActivationFunctionType.Exp
    ln_t = mybir.ActivationFunctionType.Ln
    out = {}
    for name, fns in orig.items():
        fns = set(fns)
        if name != "natural_log_exp_and_others":
            fns.discard(exp_t)
            fns.discard(ln_t)
        out[name] = fns
    return out


bacc.get_activation_tables = _patched_tables


def _host_tables():
    inv = 1.0 / (ROPE_BASE ** (np.arange(0, D, 2, dtype=np.float32) / D))
    t = np.arange(T, dtype=np.float32)
    fr = np.outer(t, inv)  # [T, 64]
    emb = np.concatenate([fr, fr], axis=-1)  # [T, D]
    cosT = np.ascontiguousarray(np.cos(emb).T).astype(BF16)  # [D, T]
    sinT = np.sin(emb).T
    sinTs = np.concatenate([-sinT[:64], sinT[64:]], axis=0)
    sinTs = np.ascontiguousarray(sinTs).astype(BF16)
    i = np.arange(P)[:, None]
    j = np.arange(512)[None, :]
    tri = (i <= j).astype(BF16)  # [128, 512] upper-triangular keep-mask
    return cosT, sinTs, tri


def _chunk_weight(w, cols):
    # [DM, cols] -> [P, NDC, cols] so the kernel's load is one contiguous DMA
    return np.ascontiguousarray(
        w.reshape(NDC, P, cols).transpose(1, 0, 2)
    ).astype(BF16)


def _kernel_body(tc, nc, xT, wq, wk, wv, wo, cosT, sinTs, tri, out):
    with (
        tc.tile_pool(name="cpool", bufs=1) as cpool,
        tc.tile_pool(name="qkvp", bufs=1) as qkvp,
        tc.tile_pool(name="wop", bufs=1) as wop,
        tc.tile_pool(name="xpool", bufs=1) as xpool,
        tc.tile_pool(name="wpool", bufs=1) as wpool,
        tc.tile_pool(name="rope", bufs=2) as rp,
        tc.tile_pool(name="psA", bufs=2, space="PSUM") as psA,
        tc.tile_pool(name="psS", bufs=2, space="PSUM") as psS,
        tc.tile_pool(name="psAV", bufs=1, space="PSUM") as psAV,
        tc.tile_pool(name="psO", bufs=1, space="PSUM") as psO,
        tc.tile_pool(name="att", bufs=2) as ap_,
        tc.tile_pool(name="expp", bufs=4) as expp,
        tc.tile_pool(name="accp", bufs=2) as accp,
        tc.tile_pool(name="agsb", bufs=1) as agsb,
        tc.tile_pool(name="dram", bufs=1, space="DRAM") as dram,
    ):
        # ---- persistent SBUF tensors ----
        tri_sb = cpool.tile([P, 512], _bf, name="tri")
        ones_sb = cpool.tile([P, P], _bf, name="ones")

        qT = qkvp.tile([P, NH, T], _bf, name="qT")
        kT = qkvp.tile([P, T], _bf, name="kT")
        v_sb = [qkvp.tile([P, D], _bf, name=f"v{tt}") for tt in range(NDC)]

        wo_sb = wop.tile([P, H, 512], _bf, name="wo_sb")
        oacc = [
            [wop.tile([P, 512], _f32, name=f"oacc{mt}_{tcn}") for tcn in range(NT)]
            for mt in range(4)
        ]

        x_sb = [
            xpool.tile([P, T], _bf, tag=f"x{dc}", name=f"x{dc}") for dc in range(NDC)
        ]
        wq_sb = wpool.tile([P, NDC, NH * D], _bf, name="wq_sb")
        wk_sb = wpool.tile([P, NDC, D], _bf, name="wk_sb")
        wv_sb = wpool.tile([P, NDC, D], _bf, name="wv_sb")
        cos_sb = wpool.tile([P, T], _bf, name="cos_sb")
        sin_sb = wpool.tile([P, T], _bf, name="sin_sb")
        ident = wpool.tile([P, P], _bf, name="ident")

        ag_in = dram.tile([NH, NT, P, 512], _bf, name="ag_in")
        ag_out = {
            (h, gi): dram.tile([KV, len(g), P, 512], _bf, name=f"ag_out{h}_{gi}")
            for h in range(NH)
            for gi, g in enumerate(GROUPS[h])
        }
        warm_in = dram.tile([P, 8], _bf, name="warm_in")
        warm_out = dram.tile([KV, P, 8], _bf, name="warm_out")

        # ---- input DMAs: x on the sync queue, weights on the scalar queue ----
        for dc in range(NDC):
            nc.sync.dma_start(x_sb[dc][:], xT[dc * P : (dc + 1) * P, :])
        nc.scalar.dma_start(wk_sb[:], wk)
        nc.scalar.dma_start(cos_sb[:], cosT)
        nc.scalar.dma_start(sin_sb[:], sinTs)
        nc.scalar.dma_start(tri_sb[:], tri)
        nc.scalar.dma_start(wv_sb[:], wv)
        nc.scalar.dma_start(wq_sb[:], wq)
        nc.scalar.dma_start(wo_sb[:], wo)
        nc.vector.memset(ones_sb[:], 1.0)
        masks.make_identity(nc, ident[:])

        # warm up the collectives path under the projection phase so the
        # first real AllGather doesn't pay first-call setup costs
        nc.gpsimd.dma_start(warm_in[:], ones_sb[:, :8])
        nc.gpsimd.collective_compute(
            "AllGather",
            mybir.AluOpType.bypass,
            replica_groups=[[0, 1, 2, 3], [4, 5, 6, 7]],
            ins=[warm_in.opt()],
            outs=[warm_out.opt()],
        )

        def rope(ps, tcn, dst):
            # RoPE: rot = src*cos + swap(src)*sin_signed.  The partition-half
            # swap must cross lanes, so it rides the GpSimd (SWDGE) DMA queue;
            # all the elementwise math is bf16 for DVE 2x mode.
            ts = slice(tcn * 512, (tcn + 1) * 512)
            src = rp.tile([P, 512], _bf, tag="rsrc", name="rsrc")
            nc.vector.tensor_copy(src[:], ps[:])
            swp = rp.tile([P, 512], _bf, tag="rswp", name="rswp")
            nc.gpsimd.dma_start(swp[0:64, :], src[64:128, :])
            nc.gpsimd.dma_start(swp[64:128, :], src[0:64, :])
            nc.vector.tensor_mul(src[:], src[:], cos_sb[:, ts])
            nc.vector.tensor_mul(swp[:], swp[:], sin_sb[:, ts])
            nc.vector.tensor_add(dst, src[:], swp[:])

        def proj_pair(lhs_of_dc, consume, t0):
            # one pair-iteration of a projection: 32 matmuls sharing each
            # stationary weight chunk across two 512-token tiles
            pss = [psA.tile([P, 512], _f32, tag="proj", name="proj") for _ in range(2)]
            for dc in range(NDC):
                lhs = lhs_of_dc(dc)
                for k, tcn in enumerate((t0, t0 + 1)):
                    nc.tensor.matmul(
                        pss[k][:],
                        lhs,
                        x_sb[dc][:, tcn * 512 : (tcn + 1) * 512],
                        start=(dc == 0),
                        stop=(dc == NDC - 1),
                    )
            for k, tcn in enumerate((t0, t0 + 1)):
                consume(tcn, pss[k])

        def consume_k(tcn, ps):
            rope(ps, tcn, kT[:, tcn * 512 : (tcn + 1) * 512])

        def consume_v(tcn, ps):
            # v^T -> PE-transpose [128,128] blocks into the [t, d] tiles AV
            # needs; transpose outputs ping-pong through the psO banks (which
            # are idle until the first AllGather consume)
            vT_sb = rp.tile([P, 512], _bf, tag="vTsb", name="vTsb")
            nc.vector.tensor_copy(vT_sb[:], ps[:])
            for sub in range(4):
                tt = tcn * 4 + sub
                ptr = psO.tile([P, P], _bf, tag=f"pos{sub % 2}", name="vtr")
                nc.tensor.transpose(ptr[:], vT_sb[:, sub * P : (sub + 1) * P], ident[:])
                nc.vector.tensor_copy(v_sb[tt][:], ptr[:])

        def make_q_consumer(j):
            def consume_q(tcn, ps):
                rope(ps, tcn, qT[:, j, tcn * 512 : (tcn + 1) * 512])

            return consume_q

        def attn_chunk(h, g):
            # scores^T [k, q] per head with causal block skip; boundary tiles
            # trimmed to their valid q-range (tri mask handles the diagonal).
            # AV runs one kt behind scores so the PE never waits on ScalarE's
            # exp round-trip.
            acc = accp.tile([P, len(g), 512], _bf, tag="acc", name=f"acc{h}")
            avs = {
                qc: psAV.tile([P, 512], _f32, tag=f"av{qc - g[0]}", name=f"av{h}_{qc}")
                for qc in g
            }
            pend_av = None
            for kt in range(4 * g[-1] + 4):
                lhs_k = kT[:, kt * P : (kt + 1) * P]
                valid = [qc for qc in g if kt <= 4 * qc + 3]
                exs = {}
                for qc in valid:
                    bound = kt // 4 == qc
                    off = 128 * (kt % 4) if bound else 0
                    w = 512 - off
                    qs = slice(qc * 512 + off, (qc + 1) * 512)
                    ps = psS.tile([P, 512], _f32, tag="s", name="s")
                    nc.tensor.matmul(
                        ps[:, :w], lhs_k, qT[:, h, qs], start=True, stop=True
                    )
                    ex = expp.tile([P, 512], _bf, tag="exp", name="exp")
                    nc.scalar.activation(
                        ex[:, :w],
                        ps[:, :w],
                        mybir.ActivationFunctionType.Exp,
                        scale=SCALE,
                    )
                    if bound:
                        nc.vector.tensor_mul(ex[:, :w], ex[:, :w], tri_sb[:, :w])
                    ai = qc - g[0]
                    if kt == 0:
                        nc.vector.tensor_copy(acc[:, ai, :], ex[:, :512])
                    else:
                        nc.vector.tensor_add(
                            acc[:, ai, off:], acc[:, ai, off:], ex[:, :w]
                        )
                    exs[qc] = (ex, off, w)
                if pend_av is not None:
                    pkt, pexs = pend_av
                    for qc, (ex, off, w) in pexs.items():
                        nc.tensor.matmul(
                            avs[qc][:, off:],
                            v_sb[pkt][:],
                            ex[:, :w],
                            start=(pkt == 0),
                            stop=(pkt == 4 * qc + 3),
                        )
                pend_av = (kt, exs)
            pkt, pexs = pend_av
            for qc, (ex, off, w) in pexs.items():
                nc.tensor.matmul(
                    avs[qc][:, off:],
                    v_sb[pkt][:],
                    ex[:, :w],
                    start=(pkt == 0),
                    stop=(pkt == 4 * qc + 3),
                )
            return acc, avs

        def attn_finish(h, gi, g, acc, avs):
            # denominators (k-partition sum + broadcast via ones matmul),
            # reciprocal as exp(-ln(x)) (same act-table set as the softmax
            # exp), normalize, ship to ag_in, AllGather the chunk
            for qc in g:
                ai = qc - g[0]
                pss = psS.tile([P, 512], _f32, tag="s", name="sden")
                nc.tensor.matmul(
                    pss[:], ones_sb[:], acc[:, ai, :], start=True, stop=True
                )
                lnv = ap_.tile([P, 512], _f32, tag="lnv", name="lnv")
                nc.scalar.activation(
                    lnv[:], pss[:], mybir.ActivationFunctionType.Ln
                )
                rec = ap_.tile([P, 512], _f32, tag="rec", name="rec")
                nc.scalar.activation(
                    rec[:], lnv[:], mybir.ActivationFunctionType.Exp, scale=-1.0
                )
                oq = ap_.tile([P, 512], _bf, tag="oq", name="oq")
                nc.vector.tensor_mul(oq[:], avs[qc][:], rec[:])
                nc.sync.dma_start(ag_in[h, qc][:, :], oq[:])

            nc.gpsimd.collective_compute(
                "AllGather",
                mybir.AluOpType.bypass,
                replica_groups=[[0, 1, 2, 3], [4, 5, 6, 7]],
                ins=[ag_in[h, g[0] : g[0] + len(g)].opt()],
                outs=[ag_out[h, gi].opt()],
            )

        def consume_chunk(h, gi, g):
            # o_proj contribution of global heads {4i+h} for this chunk's
            # t-columns, accumulated into SBUF.  One DMA per source rank.
            ag_sb = {
                i: agsb.tile([P, len(g), 512], _bf, tag=f"ag{i}", name=f"ag{h}{i}")
                for i in range(KV)
            }
            for i in range(KV):
                nc.sync.dma_start(
                    ag_sb[i][:],
                    ag_out[h, gi][i].rearrange("l p f -> p l f"),
                )
            for mt in range(4):
                pos = {
                    qc: psO.tile(
                        [P, 512], _f32, tag=f"pos{qc % 2}", name=f"pos{qc}"
                    )
                    for qc in g
                }
                for i in range(KV):
                    lhs = wo_sb[:, 4 * i + h, mt * P : (mt + 1) * P]
                    for qc in g:
                        nc.tensor.matmul(
                            pos[qc][:],
                            lhs,
                            ag_sb[i][:, qc - g[0], :],
                            start=(i == 0),
                            stop=(i == KV - 1),
                        )
                for qc in g:
                    if h == 0:
                        nc.scalar.copy(oacc[mt][qc][:], pos[qc][:])
                    else:
                        nc.vector.tensor_add(
                            oacc[mt][qc][:], oacc[mt][qc][:], pos[qc][:]
                        )
                    if h == NH - 1:
                        nc.sync.dma_start(
                            out[mt * P : (mt + 1) * P, qc * 512 : (qc + 1) * 512],
                            oacc[mt][qc][:],
                        )

        # ---- emission ----
        # k first, then v, then q0, so attention h=0 can begin right after
        for t0 in (0, 2):
            proj_pair(lambda dc: wk_sb[:, dc, :], consume_k, t0)
        for t0 in (0, 2):
            proj_pair(lambda dc: wv_sb[:, dc, :], consume_v, t0)
        for t0 in (0, 2):
            proj_pair(lambda dc: wq_sb[:, dc, 0:P], make_q_consumer(0), t0)

        pending = None
        for h in range(NH):
            units = []
            if h + 1 < NH:
                j = h + 1
                for t0 in (0, 2):
                    units.append(
                        (lambda t0=t0, j=j: proj_pair(
                            lambda dc: wq_sb[:, dc, j * P : (j + 1) * P],
                            make_q_consumer(j),
                            t0,
                        ))
                    )
            for gi, g in enumerate(GROUPS[h]):
                acc, avs = attn_chunk(h, g)
                if units:
                    units.pop(0)()
                attn_finish(h, gi, g, acc, avs)
                if pending is not None:
                    consume_chunk(*pending)
                pending = (h, gi, g)
            for u in units:
                u()
        consume_chunk(*pending)


def build_nc():
    nc = bacc.Bacc(
        "TRN2", target_bir_lowering=False, debug=False, num_devices=NCORES
    )
    xT = nc.dram_tensor("xT", [DM, T], _bf, kind="ExternalInput").ap()
    wq = nc.dram_tensor("wq", [P, NDC, NH * D], _bf, kind="ExternalInput").ap()
    wk = nc.dram_tensor("wk", [P, NDC, D], _bf, kind="ExternalInput").ap()
    wv = nc.dram_tensor("wv", [P, NDC, D], _bf, kind="ExternalInput").ap()
    wo = nc.dram_tensor("wo", [P, H, 512], _bf, kind="ExternalInput").ap()
    cosT = nc.dram_tensor("cosT", [D, T], _bf, kind="ExternalInput").ap()
    sinTs = nc.dram_tensor("sinTs", [D, T], _bf, kind="ExternalInput").ap()
    tri = nc.dram_tensor("tri", [P, 512], _bf, kind="ExternalInput").ap()
    out = nc.dram_tensor("out", [512, T], _f32, kind="ExternalOutput").ap()
    with tile.TileContext(nc) as tc:
        _kernel_body(tc, nc, xT, wq, wk, wv, wo, cosT, sinTs, tri, out)
    nc.finalize()
    return nc


def make_in_maps(x, Wq, Wk, Wv, Wo):
    cosT, sinTs, tri = _host_tables()
    xTb = [np.ascontiguousarray(x[b].T).astype(BF16) for b in range(B)]
    in_maps = []
    for r in range(NCORES):
        b, g = divmod(r, KV)
        wo_loc = Wo[:, g * 512 : (g + 1) * 512]  # [2048, 512]
        in_maps.append(
            {
                "xT": xTb[b],
                "wq": _chunk_weight(Wq[:, g * NH * D : (g + 1) * NH * D], NH * D),
                "wk": _chunk_weight(Wk[:, g * D : (g + 1) * D], D),
                "wv": _chunk_weight(Wv[:, g * D : (g + 1) * D], D),
                "wo": _chunk_weight(wo_loc, 512),
                "cosT": cosT,
                "sinTs": sinTs,
                "tri": tri,
            }
        )
    return in_maps


def assemble(results):
    out = np.empty((B, T, DM), np.float32)
    for r in range(NCORES):
        b, g = divmod(r, KV)
        out[b, :, g * 512 : (g + 1) * 512] = results[r]["out"].T
    return out


_NC_CACHE = {}


def get_nc():
    if "nc" not in _NC_CACHE:
        _NC_CACHE["nc"] = build_nc()
    return _NC_CACHE["nc"]


def run(x, Wq, Wk, Wv, Wo, trace=False, taps=False):
    nc = get_nc()
    in_maps = make_in_maps(x, Wq, Wk, Wv, Wo)
    res = run_bass_kernel_spmd(
        nc, in_maps, core_ids=list(range(NCORES)), trace=trace
    )
    return assemble(res.results), res


def kernel(x, Wq, Wk, Wv, Wo, mask=None, **_unused):
    x = np.asarray(x, dtype=np.float32)
    Wq = np.asarray(Wq, dtype=np.float32)
    Wk = np.asarray(Wk, dtype=np.float32)
    Wv = np.asarray(Wv, dtype=np.float32)
    Wo = np.asarray(Wo, dtype=np.float32)
    out, _ = run(x, Wq, Wk, Wv, Wo, trace=False)
    return out
